# revision 1
# baseline (speedup 1.0000x reference)
"""Transformer encoder layer (Informer-style) Bass/Tile kernel for TRN2. v2

v2: fused qkv+attention pipeline — q/k projections are computed per
head-pair inside the attention loop so the PE work of head-pair hp+1
overlaps the ACT-bound exp of head-pair hp. Scores are unpacked (one
K=64 matmul per head) to fit PSUM in the fused regime.

Layouts (all fp32; matmul operands dtype f32r = full PE rate):
  hsT [D,S] feature-major input -> per-hp qTh,kTh [128,S] (weights stationary)
  v_aug [S,H,65] token-major v with ones column (denominator trick)
  per head: scoresT[k,q] -> exp (ACT, no max-sub) -> probsT
            attnT[65,q] += v_aug_h^T . probsT ; row 64 = denom
            evict: DVE mult by gpsimd partition_broadcast(1/denom) -> attnC [D,S]
  o = attnC^T . woT + bo + hs -> LN1 -> x1 -> PE transpose -> x1T [D,S]
  fc1 -> hT[f,S] (gelu on ACT evict, f-groups of 512)
  fc2 -> out2[S,D] SBUF-accumulated -> LN2 -> out
"""

from contextlib import ExitStack

import concourse.bass as bass
import concourse.mybir as mybir
import concourse.tile as tile
from concourse import bacc
from concourse.masks import make_identity

AFT = mybir.ActivationFunctionType
ALU = mybir.AluOpType
F32 = mybir.dt.float32
F32R = mybir.dt.float32r

P = 128
S = 1024
D = 1024
H = 16
HD = 64
F = 4096
NTS = S // P   # 8
NTD = D // P   # 8
NTF = F // P   # 32
FG = 512       # fc1/fc2 f-group size
NFG = F // FG  # 8
FGT = FG // P  # 4
EPS = 1e-5
NCH = 2
CW = 512


def build(debug=False, sim_gelu=False):
    nc = bacc.Bacc("TRN2", target_bir_lowering=False, debug=False)

    def din(name, shape, dt=F32):
        return nc.dram_tensor(name, shape, dt, kind="ExternalInput").ap()

    io = dict(
        hsT=din("hsT", (D, S), F32R),
        hs=din("hs", (S, D)),
        wqT=din("wqT", (D, D), F32R),   # wq.T * SCALING
        wkT=din("wkT", (D, D), F32R),
        wvT=din("wvT", (D, D), F32R),
        woT=din("woT", (D, D), F32R),
        bq=din("bq", (D,)),             # * SCALING
        bk=din("bk", (D,)),
        bv=din("bv", (D,)),
        bo=din("bo", (D,)),
        g1=din("g1", (D,)),
        b1=din("b1", (D,)),
        g2=din("g2", (D,)),
        b2=din("b2", (D,)),
        f1w=din("f1w", (D, F), F32R),   # fc1_w.T
        f1b=din("f1b", (F,)),
        f2w=din("f2w", (F, D), F32R),   # fc2_w.T
        f2b=din("f2b", (D,)),
        out=nc.dram_tensor("out", (S, D), F32, kind="ExternalOutput").ap(),
        x1_dram=nc.dram_tensor("x1_spill", (S, D), F32, kind="Internal").ap(),
    )

    dbg = {}
    if debug:
        for nm, shp in [("dbg_qT", (D, S)), ("dbg_kT", (D, S)),
                        ("dbg_vaug", (S, H * (HD + 1))),
                        ("dbg_attnC", (D, S)), ("dbg_x1", (S, D)),
                        ("dbg_out2", (S, D))]:
            dbg[nm] = nc.dram_tensor(nm, shp, F32, kind="ExternalOutput").ap()
    io["dbg"] = dbg
    io["debug"] = debug
    io["sim_gelu"] = sim_gelu

    with tile.TileContext(nc) as tc:
        _body(tc, io)
    nc.compile()
    return nc


def _body(tc, t):
    nc = tc.nc
    hsT, hs = t["hsT"], t["hs"]
    wqT, wkT, wvT, woT = t["wqT"], t["wkT"], t["wvT"], t["woT"]
    bq, bk, bv, bo = t["bq"], t["bk"], t["bv"], t["bo"]
    g1, b1, g2, b2 = t["g1"], t["b1"], t["g2"], t["b2"]
    f1w, f1b, f2w, f2b = t["f1w"], t["f1b"], t["f2w"], t["f2b"]
    out, x1_dram, dbg, debug = t["out"], t["x1_dram"], t["dbg"], t["debug"]
    sim_gelu = t["sim_gelu"]

    const = tc.alloc_tile_pool(name="const", bufs=1)

    bqk_t = const.tile([P, 2, NTD], F32)
    nc.sync.dma_start(out=bqk_t[:, 0, :], in_=bq.rearrange("(t p) -> p t", p=P))
    nc.sync.dma_start(out=bqk_t[:, 1, :], in_=bk.rearrange("(t p) -> p t", p=P))
    f1b_t = const.tile([P, NTF], F32)
    nc.sync.dma_start(out=f1b_t, in_=f1b.rearrange("(t p) -> p t", p=P))
    eps_t = const.tile([P, 1], F32)
    nc.vector.memset(eps_t, EPS)
    ones_t = const.tile([P, 1], F32)
    nc.vector.memset(ones_t, 1.0)
    identity = const.tile([P, P], F32)
    make_identity(nc, identity)

    def bcast_tile(pool, src):
        bt = pool.tile([P, D], F32, tag="bc" + src.name, name="bc" + src.name)
        nc.sync.dma_start(out=bt, in_=src.unsqueeze(0).broadcast_to((P, D)))
        return bt

    # big pool: hsT -> attnC -> x1T share two 4MB slots across the kernel
    big = tc.alloc_tile_pool(name="big", bufs=2)
    hsT_sb = big.tile([P, NTD, S], F32R, tag="big")
    # wo preloaded during attention so out-proj starts immediately after
    wo_pre = tc.alloc_tile_pool(name="wo_pre", bufs=1)
    wo_sb = wo_pre.tile([P, NTD, D], F32R)
    vaug_pool = tc.alloc_tile_pool(name="vaug_pool", bufs=1)
    v_aug = vaug_pool.tile([P, NTS, H, HD + 1], F32R)

    for td in range(NTD):
        nc.sync.dma_start(out=hsT_sb[:, td, :], in_=hsT[td * P:(td + 1) * P, :])
    nc.vector.tensor_copy(
        out=v_aug[:, :, :, HD:HD + 1],
        in_=ones_t.unsqueeze(1).unsqueeze(1).broadcast_to((P, NTS, H, 1)))

    attnC = big.tile([P, NTD, S], F32R, tag="big")

    # single psum pools for the whole kernel (no phase-boundary deps)
    psU = tc.alloc_tile_pool(name="psU", bufs=3, space="PSUM")
    psQ = tc.alloc_tile_pool(name="psQ", bufs=2, space="PSUM")

    # ---------------- fused qkv + attention ----------------
    if True:
        # v projection (hsT stationary, wvT moving) -> token-major v_aug
        with tc.tile_pool(name="pv", bufs=1) as pv_pool:
            bv_bc = bcast_tile(pv_pool, bv)
            wv_sb = pv_pool.tile([P, NTD, D], F32R)
            for ti in range(NTD):
                nc.sync.dma_start(out=wv_sb[:, ti, :],
                                  in_=wvT[ti * P:(ti + 1) * P, :])
            for ts in range(NTS):
                ps = psU.tile([P, D], F32, tag="u", name="psv")
                for nch in range(NCH):
                    for ti in range(NTD):
                        nc.tensor.matmul(
                            ps[:, nch * CW:(nch + 1) * CW],
                            lhsT=hsT_sb[:, ti, ts * P:(ts + 1) * P],
                            rhs=wv_sb[:, ti, nch * CW:(nch + 1) * CW],
                            start=(ti == 0), stop=(ti == NTD - 1))
                nc.vector.tensor_tensor(
                    out=v_aug[:, ts, :, 0:HD],
                    in0=ps.rearrange("p (h e) -> p h e", h=H),
                    in1=bv_bc.rearrange("p (h e) -> p h e", h=H),
                    op=ALU.add)

        for ti in range(NTD):
            nc.sync.dma_start(out=wo_sb[:, ti, :], in_=woT[ti * P:(ti + 1) * P, :])

        fused = ExitStack()
        qk_pool = fused.enter_context(tc.tile_pool(name="qkt", bufs=2))
        wqk_pool = fused.enter_context(tc.tile_pool(name="wqkp", bufs=2))
        probs_pool = fused.enter_context(tc.tile_pool(name="probs", bufs=4))
        bc_pool = fused.enter_context(tc.tile_pool(name="bcp", bufs=2))
        rr_pool = fused.enter_context(tc.tile_pool(name="rrp", bufs=1))
        stg_pool = fused.enter_context(tc.tile_pool(name="stgp", bufs=2))

        def qk_proj(hp):
            qkh = {}
            for wsrc, bidx, nm in ((wqT, 0, "q"), (wkT, 1, "k")):
                wblk = wqk_pool.tile([P, NTD, P], F32R, tag="w" + nm, name="w" + nm)
                for ti in range(NTD):
                    nc.sync.dma_start(
                        out=wblk[:, ti, :],
                        in_=wsrc[ti * P:(ti + 1) * P, hp * P:(hp + 1) * P])
                dst = qk_pool.tile([P, S], F32R, tag=nm + "T", name=nm + "Th")
                for nch in range(NCH):
                    ps = psQ.tile([P, CW], F32, tag="q5", name="psq")
                    for ti in range(NTD):
                        nc.tensor.matmul(
                            ps,
                            lhsT=wblk[:, ti, :],
                            rhs=hsT_sb[:, ti, nch * CW:(nch + 1) * CW],
                            start=(ti == 0), stop=(ti == NTD - 1))
                    nc.vector.tensor_scalar_add(
                        out=dst[:, nch * CW:(nch + 1) * CW], in0=ps,
                        scalar1=bqk_t[:, bidx, hp:hp + 1])
                qkh[nm] = dst
            if debug:
                nc.gpsimd.dma_start(out=dbg["dbg_qT"][hp * P:(hp + 1) * P, :], in_=qkh["q"])
                nc.gpsimd.dma_start(out=dbg["dbg_kT"][hp * P:(hp + 1) * P, :], in_=qkh["k"])
            return qkh

        for hp in range(H // 2):
            qkh = qk_proj(hp)

            for h in (2 * hp, 2 * hp + 1):
                r0 = (h % 2) * HD
                last_hp = (hp == H // 2 - 1)
                if last_hp:
                    ps_at = [psQ.tile([P, CW], F32, tag="q5", name=f"atq{h}{c}")
                             for c in range(NCH)]
                else:
                    at_full = psU.tile([P, S], F32, tag="u", name=f"at{h}")
                    ps_at = [at_full[:, c * CW:(c + 1) * CW] for c in range(NCH)]
                for tk in range(NTS):
                    ps_sc = psU.tile([P, S], F32, tag="u", name=f"sc{h}")
                    for nch in range(NCH):
                        nc.tensor.matmul(
                            ps_sc[:, nch * CW:(nch + 1) * CW],
                            lhsT=qkh["k"][r0:r0 + HD, tk * P:(tk + 1) * P],
                            rhs=qkh["q"][r0:r0 + HD, nch * CW:(nch + 1) * CW],
                            start=True, stop=True)
                    pr = probs_pool.tile([P, S], F32R, tag="pr", name=f"pr{h}")
                    nc.scalar.activation(out=pr, in_=ps_sc, func=AFT.Exp)
                    for nch in range(NCH):
                        nc.tensor.matmul(
                            ps_at[nch][0:HD + 1, :],
                            lhsT=v_aug[:, tk, h, :],
                            rhs=pr[:, nch * CW:(nch + 1) * CW],
                            start=(tk == 0), stop=(tk == NTS - 1))
                stg = stg_pool.tile([P, S], F32, tag="stg", name=f"stg{h}")
                for nch in range(NCH):
                    nc.vector.tensor_copy(
                        out=stg[0:HD + 1, nch * CW:(nch + 1) * CW],
                        in_=ps_at[nch][0:HD + 1, :])
                rrow = rr_pool.tile([1, S], F32, tag="rr", name=f"rr{h}")
                nc.vector.reciprocal(out=rrow, in_=stg[HD:HD + 1, :])
                bc = bc_pool.tile([P, S], F32, tag="bc", name=f"bcr{h}")
                nc.gpsimd.partition_broadcast(out_ap=bc, in_ap=rrow)
                nc.vector.tensor_tensor(
                    out=attnC[r0:r0 + HD, hp, :],
                    in0=stg[0:HD, :], in1=bc[0:HD, :], op=ALU.mult)
        fused.close()

    if debug:
        for ts in range(NTS):
            nc.gpsimd.dma_start(
                out=dbg["dbg_vaug"][ts * P:(ts + 1) * P, :],
                in_=v_aug[:, ts, :, :])
        for td in range(NTD):
            nc.gpsimd.dma_start(out=dbg["dbg_attnC"][td * P:(td + 1) * P, :], in_=attnC[:, td, :])

    vaug_pool.release()

    # ---------------- out proj + residual + LN1 + transpose ----------------
    x1T_sb = big.tile([P, NTD, S], F32R, tag="big")

    with tc.tile_pool(name="phaseC", bufs=1) as pc_pool, \
         tc.tile_pool(name="tmpC", bufs=3) as tmpC:
        hs_sb = pc_pool.tile([P, NTS, D], F32)
        hs_r = hs.rearrange("(t p) d -> p t d", p=P)
        for ts in range(NTS):
            nc.sync.dma_start(out=hs_sb[:, ts, :], in_=hs_r[:, ts, :])
        bo_bc = bcast_tile(pc_pool, bo)
        g1_bc = bcast_tile(pc_pool, g1)
        b1_bc = bcast_tile(pc_pool, b1)
        for ts in range(NTS):
            nc.vector.tensor_tensor(out=hs_sb[:, ts, :], in0=hs_sb[:, ts, :],
                                    in1=bo_bc, op=ALU.add)
        x1_r = x1_dram.rearrange("(t p) d -> p t d", p=P)
        for ts in range(NTS):
            ps = psU.tile([P, D], F32, tag="u", name="pso")
            for nch in range(NCH):
                for td in range(NTD):
                    nc.tensor.matmul(
                        ps[:, nch * CW:(nch + 1) * CW],
                        lhsT=attnC[:, td, ts * P:(ts + 1) * P],
                        rhs=wo_sb[:, td, nch * CW:(nch + 1) * CW],
                        start=(td == 0), stop=(td == NTD - 1))
            x0 = tmpC.tile([P, D], F32, tag="x0", name="x0")
            nc.vector.tensor_tensor(out=x0, in0=ps, in1=hs_sb[:, ts, :], op=ALU.add)
            x1t = tmpC.tile([P, D], F32, tag="x1t", name="x1t")
            _layernorm(nc, tmpC, x1t, x0, g1_bc, b1_bc, eps_t,
                       badd_gpsimd=(ts % 2 == 0))
            nc.sync.dma_start(out=x1_r[:, ts, :], in_=x1t)
            if debug:
                nc.sync.dma_start(
                    out=dbg["dbg_x1"].rearrange("(t p) d -> p t d", p=P)[:, ts, :],
                    in_=x1t)
            for td in range(NTD):
                pst = psQ.tile([P, P], F32, tag="q5", name="pst")
                nc.tensor.transpose(pst, x1t[:, td * P:(td + 1) * P], identity)
                nc.any.tensor_copy(out=x1T_sb[:, td, ts * P:(ts + 1) * P], in_=pst)

    wo_pre.release()

    # ---------------- FFN ----------------
    out2_pool = tc.alloc_tile_pool(name="out2_pool", bufs=1)
    out2 = out2_pool.tile([P, NTS, D], F32)

    wbufs = 1 if sim_gelu else 2
    with tc.tile_pool(name="f1wp", bufs=1) as f1wp, \
         tc.tile_pool(name="f2wp", bufs=wbufs) as f2wp, \
         tc.tile_pool(name="hTp", bufs=2 if not sim_gelu else 1) as hTp, \
         tc.tile_pool(name="fcb", bufs=1) as fcb_pool, \
         tc.tile_pool(name="tmpE", bufs=2) as tmpE:
        f2b_bc = bcast_tile(fcb_pool, f2b)
        g2_bc = bcast_tile(fcb_pool, g2)
        b2_bc = bcast_tile(fcb_pool, b2)
        x1_r2 = x1_dram.rearrange("(t p) d -> p t d", p=P)
        out_r = out.rearrange("(t p) d -> p t d", p=P)
        for g in range(NFG):
            w1 = f1wp.tile([P, NTD, FG], F32R, tag="w1", name="w1")
            for td in range(NTD):
                nc.sync.dma_start(out=w1[:, td, :],
                                  in_=f1w[td * P:(td + 1) * P, g * FG:(g + 1) * FG])
            w2 = f2wp.tile([P, FGT, D], F32R, tag="w2", name="w2")
            for ft in range(FGT):
                tf = g * FGT + ft
                nc.sync.dma_start(out=w2[:, ft, :], in_=f2w[tf * P:(tf + 1) * P, :])
            hT_g = hTp.tile([P, FGT, S], F32R, tag="hT", name="hT_g")
            for ft in range(FGT):
                tf = g * FGT + ft
                ps = psU.tile([P, S], F32, tag="u", name="psh")
                for nch in range(NCH):
                    for td in range(NTD):
                        nc.tensor.matmul(
                            ps[:, nch * CW:(nch + 1) * CW],
                            lhsT=w1[:, td, ft * P:(ft + 1) * P],
                            rhs=x1T_sb[:, td, nch * CW:(nch + 1) * CW],
                            start=(td == 0), stop=(td == NTD - 1))
                if not sim_gelu:
                    nc.scalar.activation(out=hT_g[:, ft, :], in_=ps, func=AFT.Gelu,
                                         bias=f1b_t[:, tf:tf + 1], scale=1.0)
                else:
                    xg = hTp.tile([P, S], F32, tag="xg", name="xg")
                    nc.scalar.activation(out=xg, in_=ps, func=AFT.Identity,
                                         bias=f1b_t[:, tf:tf + 1], scale=1.0)
                    sg = hTp.tile([P, S], F32, tag="sg", name="sg")
                    nc.scalar.activation(out=sg, in_=xg, func=AFT.Sigmoid,
                                         bias=0.0, scale=1.702)
                    nc.vector.tensor_tensor(out=hT_g[:, ft, :], in0=xg, in1=sg,
                                            op=ALU.mult)
            for ts in range(NTS):
                ps = psU.tile([P, D], F32, tag="u", name="pso2")
                for nch in range(NCH):
                    for ft in range(FGT):
                        nc.tensor.matmul(
                            ps[:, nch * CW:(nch + 1) * CW],
                            lhsT=hT_g[:, ft, ts * P:(ts + 1) * P],
                            rhs=w2[:, ft, nch * CW:(nch + 1) * CW],
                            start=(ft == 0), stop=(ft == FGT - 1))
                if g == 0:
                    nc.vector.tensor_tensor(out=out2[:, ts, :], in0=ps,
                                            in1=f2b_bc, op=ALU.add)
                else:
                    nc.vector.tensor_tensor(out=out2[:, ts, :], in0=ps,
                                            in1=out2[:, ts, :], op=ALU.add)
                if g == NFG - 2:
                    # fold the x1 residual in now (DVE is idle in g7's fc1)
                    x1t = tmpE.tile([P, D], F32, tag="x1e", name="x1e")
                    nc.sync.dma_start(out=x1t, in_=x1_r2[:, ts, :])
                    nc.vector.tensor_tensor(out=out2[:, ts, :],
                                            in0=out2[:, ts, :], in1=x1t,
                                            op=ALU.add)
                if g == NFG - 1:
                    # inline LN2 as each out2[ts] completes
                    if debug:
                        nc.sync.dma_start(
                            out=dbg["dbg_out2"].rearrange(
                                "(t p) d -> p t d", p=P)[:, ts, :],
                            in_=out2[:, ts, :])
                    yt = tmpE.tile([P, D], F32, tag="ye", name="ye")
                    _layernorm(nc, tmpE, yt, out2[:, ts, :], g2_bc, b2_bc,
                               eps_t, badd_gpsimd=(ts % 2 == 0))
                    nc.sync.dma_start(out=out_r[:, ts, :], in_=yt)

    out2_pool.release()
    psQ.release()
    psU.release()
    big.release()
    const.release()


def _layernorm(nc, pool, out_t, x0, g_bc, b_bc, eps_t, badd_gpsimd=False):
    """out = (x0 - mean)/sqrt(var+eps) * g + b   (mean/var along free dim D)"""
    stats = pool.tile([P, 2, 6], F32, tag="lnstats", name="lnstats")
    nc.vector.bn_stats(out=stats[:, 0, :], in_=x0[:, 0:512])
    nc.vector.bn_stats(out=stats[:, 1, :], in_=x0[:, 512:1024])
    mv = pool.tile([P, 2], F32, tag="lnmv", name="lnmv")
    nc.vector.bn_aggr(out=mv, in_=stats)
    nc.scalar.activation(out=mv[:, 1:2], in_=mv[:, 1:2], func=AFT.Sqrt,
                         bias=eps_t, scale=1.0)
    nc.vector.reciprocal(out=mv[:, 1:2], in_=mv[:, 1:2])
    # bneg = -mean * rstd, then xhat = x0*rstd + bneg on ACT (idle engine)
    bneg = pool.tile([P, 1], F32, tag="lnbneg", name="lnbneg")
    nc.vector.tensor_scalar(out=bneg, in0=mv[:, 0:1], scalar1=mv[:, 1:2],
                            scalar2=-1.0, op0=ALU.mult, op1=ALU.mult)
    nc.scalar.activation(out=out_t, in_=x0, func=AFT.Identity,
                         bias=bneg, scale=mv[:, 1:2])
    nc.gpsimd.tensor_tensor(out=out_t, in0=out_t, in1=g_bc, op=ALU.mult)
    badd = nc.gpsimd if badd_gpsimd else nc.vector
    badd.tensor_tensor(out=out_t, in0=out_t, in1=b_bc, op=ALU.add)


# ---------------------------------------------------------------------------
# Full-input entry point: data-parallel over batch across 8 NeuronCores.
# ---------------------------------------------------------------------------
import numpy as np
from concourse import bass_utils

B = 8
SCALING = HD ** -0.5

_NC_CACHE = None


def _get_nc():
    global _NC_CACHE
    if _NC_CACHE is None:
        _NC_CACHE = build(debug=False)
    return _NC_CACHE


def _prep_core_inputs(b_hs, w):
    c = np.ascontiguousarray
    f = np.float32

    def a(x):
        return c(np.asarray(x)).astype(f, copy=False)

    return {
        "hsT": a(b_hs.T),
        "hs": a(b_hs),
        "wqT": a(np.asarray(w["wq"]).T * SCALING),
        "wkT": a(np.asarray(w["wk"]).T),
        "wvT": a(np.asarray(w["wv"]).T),
        "woT": a(np.asarray(w["wo"]).T),
        "bq": a(np.asarray(w["bq"]) * SCALING),
        "bk": a(w["bk"]),
        "bv": a(w["bv"]),
        "bo": a(w["bo"]),
        "g1": a(w["ln1_g"]),
        "b1": a(w["ln1_b"]),
        "g2": a(w["ln2_g"]),
        "b2": a(w["ln2_b"]),
        "f1w": a(np.asarray(w["fc1_w"]).T),
        "f1b": a(w["fc1_b"]),
        "f2w": a(np.asarray(w["fc2_w"]).T),
        "f2b": a(w["fc2_b"]),
    }


def kernel(**inputs):
    """Takes full unsharded inputs (setup_inputs() keys), returns [B, S, D]."""
    w = {k: np.asarray(v) for k, v in inputs.items()}
    hs_all = w["hidden_states"]
    assert hs_all.shape == (B, S, D), hs_all.shape
    nc = _get_nc()
    in_maps = [_prep_core_inputs(hs_all[c], w) for c in range(B)]
    res = bass_utils.run_bass_kernel_spmd(nc, in_maps, core_ids=list(range(B)))
    out_full = np.stack([res.results[c]["out"] for c in range(B)])
    return out_full.astype(np.float32, copy=False)



# revision 21
# speedup vs baseline: 1.2375x; 1.2375x over previous
"""Transformer encoder layer (Informer-style) Bass/Tile kernel for TRN2. v3

v3: mixed-precision redesign.
  - Attention GEMMs (q/k/v proj, attnV, out-proj) run in fp8e4 (e4m3) with
    MatmulPerfMode.DoubleRow: each instruction contracts 2x128 K at 0.5
    cycles/row -> 4x the fp32r FLOP rate.
  - Scores (K=64 per head) stay in bf16 at 1.0 cyc/row (DoubleRow would need
    a 32-partition layout).
  - FFN stays 16-bit (bf16) for precision (fp8 FFN busts the 2e-2 budget);
    fc2 accumulates in PSUM per token tile over each half of F, with hT
    half-resident in SBUF -> no SBUF out2 accumulation chain, short tail.
  - Host pre-quantizes weights/hsT and folds bo into the residual stream.

Layouts:
  hsT8 [D,S] fp8 feature-major input; hs [S,D] fp32 (+bo folded)
  per hp: qk_sb [P, hp, {q,k}, S] bf16 (weights stationary fp8 DoubleRow)
  v_aug [P, ts, H, 65] fp8 token-major v with ones column (denominator trick)
  per head: scoresT[k,q] psum -> exp (ACT, no max-sub) -> probs fp8 pairs
            attnT[65,q] += v_aug_h^T . probs (DoubleRow over tk pairs)
            evict: DVE recip+bcast+mult -> attnC [P, td, S] fp8
  out-proj fp8 DoubleRow + residual + LN1 -> x1 bf16 -> PE transpose -> x1T
  fc1 bf16 -> hT[f,S] half-resident (gelu on ACT evict)
  fc2 bf16 psum-accumulated per ts per half -> +residual -> LN2 -> out
"""

from contextlib import ExitStack

import concourse.bass as bass
import concourse.mybir as mybir
import concourse.tile as tile
from concourse import bacc
from concourse.masks import make_identity

AFT = mybir.ActivationFunctionType
ALU = mybir.AluOpType
F32 = mybir.dt.float32
F32R = mybir.dt.float32r
BF16 = mybir.dt.bfloat16
FP8 = mybir.dt.float8e4
DR = mybir.MatmulPerfMode.DoubleRow

P = 128
S = 1024
D = 1024
H = 16
HD = 64
F = 4096
NTS = S // P   # 8
NTD = D // P   # 8
NTF = F // P   # 32
EPS = 1e-5
NCH = 2
CW = 512
NFH = NTF // 2  # 16 f-tiles per FFN half
NW1G = 8        # fc1 weight stream groups (512 f-dims each)
W1GT = NTF // NW1G  # 4 f-tiles per w1 group
NFQ = 4         # fc2 F quarters
FQT = NTF // NFQ    # 8 f-tiles per quarter


def build(debug=False):
    nc = bacc.Bacc("TRN2", target_bir_lowering=False, debug=False)

    def din(name, shape, dt=F32):
        return nc.dram_tensor(name, shape, dt, kind="ExternalInput").ap()

    io = dict(
        hsT8=din("hsT8", (P, NTD, S), FP8),       # [p, td, s] pretiled
        hs=din("hs", (P, NTS, D)),                # [p, ts, d], + bo folded
        wqk8=din("wqk8", (P, H // 2, 2, NTD, P), FP8),  # [p, hp, qk, ti, c]
        wv8=din("wv8", (P, NTD, D), FP8),         # [p, ti, c] (wv.T tiled)
        wo8=din("wo8", (P, NTD, D), FP8),
        bqk=din("bqk", (2, D)),          # row0: bq*SCALING, row1: bk
        bv=din("bv", (D,)),
        g1=din("g1", (D,)),
        b1=din("b1", (D,)),
        g2=din("g2", (D,)),
        b2=din("b2", (D,)),
        f1w=din("f1w", (NW1G, P, NTD, W1GT * P), BF16),  # [g, p, td, c]
        f1b=din("f1b", (F,)),
        f2w=din("f2w", (NFQ, P, FQT, D), BF16),   # [quarter, p, ft, c]
        f2b=din("f2b", (D,)),
        out=nc.dram_tensor("out", (S, D), F32, kind="ExternalOutput").ap(),
    )

    dbg = {}
    if debug:
        for nm, shp in [("dbg_qT", (D, S)), ("dbg_kT", (D, S)),
                        ("dbg_vaug", (S, H * (HD + 1))),
                        ("dbg_attnC", (D, S)), ("dbg_x1", (S, D)),
                        ("dbg_out2", (S, D))]:
            dbg[nm] = nc.dram_tensor(nm, shp, F32, kind="ExternalOutput").ap()
    io["dbg"] = dbg
    io["debug"] = debug

    with tile.TileContext(nc) as tc:
        _body(tc, io)
    nc.compile()
    return nc



def _body(tc, t):
    nc = tc.nc
    hsT8, hs = t["hsT8"], t["hs"]
    wqk8, wv8, wo8 = t["wqk8"], t["wv8"], t["wo8"]
    bqk, bv = t["bqk"], t["bv"]
    g1, b1, g2, b2 = t["g1"], t["b1"], t["g2"], t["b2"]
    f1w, f1b, f2w, f2b = t["f1w"], t["f1b"], t["f2w"], t["f2b"]
    out, dbg, debug = t["out"], t["dbg"], t["debug"]

    NQG = 2            # query groups
    QW = S // NQG      # 512 queries per group
    QTS = NTS // NQG   # 4 token tiles per group
    NPAIR = NTS // 2   # 4 tk pairs
    NFQ = 4            # F quarters
    FQT = NTF // NFQ   # 8 f-tiles per quarter
    NW1G_ = 8          # w1 stream groups (512 f each)
    W1GT_ = NTF // NW1G_  # 4 f-tiles per w1 group

    const = tc.alloc_tile_pool(name="const", bufs=1)
    bqk_t = const.tile([P, 2, NTD], F32)
    f1b_t = const.tile([P, NTF], F32)
    eps_t = const.tile([P, 1], F32)
    nc.vector.memset(eps_t, EPS)

    def bcast_tile(pool, src, nm, eng=None):
        bt = pool.tile([P, D], F32, tag="bc" + nm, name="bc" + nm)
        (eng or nc.sync).dma_start(out=bt,
                                   in_=src.unsqueeze(0).broadcast_to((P, D)))
        return bt

    # --- persistent SBUF (right side, reverse lifetime order) ---
    in8c = tc.alloc_tile_pool(name="in8c", bufs=1, side="right")
    wo_sb = in8c.tile([P, NTD, D], FP8)
    attnC = in8c.tile([P, NTD, S], FP8)
    in8b = tc.alloc_tile_pool(name="in8b", bufs=1, side="right")
    v_aug = in8b.tile([P, NTS, H, HD + 1], FP8)
    qk_sb = in8b.tile([P, H // 2, 2, S], FP8)  # [p, hp, {q,k}, s]
    in8a = tc.alloc_tile_pool(name="in8a", bufs=1, side="right")
    hsT_sb = in8a.tile([P, NTD, S], FP8)
    wv_sb = in8a.tile([P, NTD, D], FP8)
    wqk_sb = in8a.tile([P, H // 2, 2, NTD, P], FP8)

    nc.sync.dma_start(out=hsT_sb, in_=hsT8)
    nc.sync.dma_start(out=wqk_sb, in_=wqk8)
    nc.scalar.dma_start(out=wv_sb, in_=wv8)
    nc.scalar.dma_start(out=bqk_t[:, 0, :],
                        in_=bqk[0].rearrange("(t p) -> p t", p=P))
    nc.scalar.dma_start(out=bqk_t[:, 1, :],
                        in_=bqk[1].rearrange("(t p) -> p t", p=P))
    nc.scalar.dma_start(out=wo_sb, in_=wo8)
    nc.scalar.dma_start(out=f1b_t, in_=f1b.rearrange("(t p) -> p t", p=P))
    nc.vector.memset(v_aug[:, :, :, HD:HD + 1], 1.0)

    x16 = tc.alloc_tile_pool(name="x16", bufs=1)
    x1_sb = x16.tile([P, NTS, D], BF16)   # token-major (residual2)
    x1T_sb = x16.tile([P, NTD, S], BF16)  # feature-major (fc1 rhs)

    # psA: shared [128,1024]-class psums (qk/v/scores/out-proj/fc2)
    psA = tc.alloc_tile_pool(name="psA", bufs=3, space="PSUM")
    # psB: attnV accumulators [65, QW]
    psB = tc.alloc_tile_pool(name="psB", bufs=2, space="PSUM")

    pv_pool = tc.alloc_tile_pool(name="pv", bufs=1, side="right")
    bv_bc = bcast_tile(pv_pool, bv, "bv")

    def v_proj(ts):
        ps = psA.tile([P, D], F32, tag="u", name="psv")
        for nch in range(NCH):
            for i in range(NTD // 2):
                nc.tensor.matmul(
                    ps[:, nch * CW:(nch + 1) * CW],
                    lhsT=hsT_sb[:, 2 * i:2 * i + 2, ts * P:(ts + 1) * P],
                    rhs=wv_sb[:, 2 * i:2 * i + 2, nch * CW:(nch + 1) * CW],
                    start=(i == 0), stop=(i == NTD // 2 - 1),
                    perf_mode=DR)
        nc.vector.tensor_tensor(
            out=v_aug[:, ts, :, 0:HD],
            in0=ps.rearrange("p (h e) -> p h e", h=H),
            in1=bv_bc.rearrange("p (h e) -> p h e", h=H),
            op=ALU.add)

    def qk_proj(hp, bidx):
        ps = psA.tile([P, S], F32, tag="u", name="psqk")
        for nch in range(NCH):
            for i in range(NTD // 2):
                nc.tensor.matmul(
                    ps[:, nch * CW:(nch + 1) * CW],
                    lhsT=wqk_sb[:, hp, bidx, 2 * i:2 * i + 2, :],
                    rhs=hsT_sb[:, 2 * i:2 * i + 2, nch * CW:(nch + 1) * CW],
                    start=(i == 0), stop=(i == NTD // 2 - 1),
                    perf_mode=DR)
        nc.vector.tensor_scalar_add(
            out=qk_sb[:, hp, bidx, :], in0=ps,
            scalar1=bqk_t[:, bidx, hp:hp + 1])

    probs_pool = tc.alloc_tile_pool(name="probs", bufs=4)
    bc_pool = tc.alloc_tile_pool(name="bcp", bufs=2)
    rr_pool = tc.alloc_tile_pool(name="rrp", bufs=2)

    def scores_head(h, qg):
        """scores + exp for one head on query group qg; returns probs tiles."""
        hp, r0 = h // 2, (h % 2) * HD
        qh = qk_sb[:, hp, 0, :]
        kh = qk_sb[:, hp, 1, :]
        prs = []
        for j in range(NPAIR):
            psc = psA.tile([P, 2, QW], F32, tag="u", name=f"sc{h}q{qg}")
            for u in range(2):
                tk = 2 * j + u
                nc.tensor.matmul(
                    psc[:, u, :],
                    lhsT=kh[r0:r0 + HD, tk * P:(tk + 1) * P],
                    rhs=qh[r0:r0 + HD, qg * QW:(qg + 1) * QW],
                    start=True, stop=True)
            prj = probs_pool.tile([P, 2, QW], FP8, tag="pr", name=f"pr{h}q{qg}")
            nc.scalar.activation(out=prj, in_=psc, func=AFT.Exp)
            prs.append(prj)
        return prs

    def attnv_head(h, qg, prs):
        hp, r0 = h // 2, (h % 2) * HD
        at = psB.tile([HD + 1, QW], F32, tag="av", name=f"at{h}q{qg}")
        for j in range(NPAIR):
            nc.tensor.matmul(
                at,
                lhsT=v_aug[:, 2 * j:2 * j + 2, h, :],
                rhs=prs[j],
                start=(j == 0), stop=(j == NPAIR - 1),
                perf_mode=DR)
        rrow = rr_pool.tile([1, QW], F32, tag="rr", name=f"rr{h}")
        nc.vector.reciprocal(out=rrow, in_=at[HD:HD + 1, :])
        bc = bc_pool.tile([HD, QW], F32, tag="bc", name=f"bcr{h}")
        nc.gpsimd.partition_broadcast(out_ap=bc, in_ap=rrow)
        nc.vector.tensor_tensor(
            out=attnC[r0:r0 + HD, hp, qg * QW:(qg + 1) * QW],
            in0=at[0:HD, :], in1=bc, op=ALU.mult)

    # ---------------- phase B': fused qkv-proj + attention on qg0 ----------
    qk_proj(0, 0)
    qk_proj(0, 1)
    pend = None
    for h in range(H):
        if h == 1:
            for ts in range(NTS):
                v_proj(ts)
        if h % 2 == 1 and h // 2 + 1 < H // 2:
            qk_proj(h // 2 + 1, 0)
            qk_proj(h // 2 + 1, 1)
        prs = scores_head(h, 0)
        if pend is not None:
            attnv_head(pend[0], 0, pend[1])
        pend = (h, prs)
    attnv_head(pend[0], 0, pend[1])
    pv_pool.release()
    in8a.release()

    if debug:
        for hp in range(H // 2):
            nc.gpsimd.dma_start(out=dbg["dbg_qT"][hp * P:(hp + 1) * P, :],
                                in_=qk_sb[:, hp, 0, :])
            nc.gpsimd.dma_start(out=dbg["dbg_kT"][hp * P:(hp + 1) * P, :],
                                in_=qk_sb[:, hp, 1, :])
        for ts in range(NTS):
            nc.gpsimd.dma_start(
                out=dbg["dbg_vaug"][ts * P:(ts + 1) * P, :],
                in_=v_aug[:, ts, :, :])

    # ---------------- shared LN1 / out-proj / FFN machinery ----------------
    lnp = tc.alloc_tile_pool(name="lnp", bufs=1)
    g1_bc = bcast_tile(lnp, g1, "g1", eng=nc.scalar)
    b1_bc = bcast_tile(lnp, b1, "b1", eng=nc.scalar)
    g2_bc = bcast_tile(lnp, g2, "g2", eng=nc.scalar)
    b2_bc = bcast_tile(lnp, b2, "b2", eng=nc.scalar)
    f2b_bc = bcast_tile(lnp, f2b, "f2b", eng=nc.scalar)

    hs_pool = tc.alloc_tile_pool(name="hsp", bufs=2)
    tmpC = tc.alloc_tile_pool(name="tmpC", bufs=2)
    out_r = out.rearrange("(t p) d -> p t d", p=P)

    def out_proj_ln1(ts):
        """out-proj for token tile ts + residual + LN1 + transpose to x1T."""
        hst = hs_pool.tile([P, D], F32, tag="hs", name=f"hs{ts}")
        nc.gpsimd.dma_start(out=hst, in_=hs[:, ts, :])
        ps = psA.tile([P, D], F32, tag="u", name="pso")
        for nch in range(NCH):
            for i in range(NTD // 2):
                nc.tensor.matmul(
                    ps[:, nch * CW:(nch + 1) * CW],
                    lhsT=attnC[:, 2 * i:2 * i + 2, ts * P:(ts + 1) * P],
                    rhs=wo_sb[:, 2 * i:2 * i + 2, nch * CW:(nch + 1) * CW],
                    start=(i == 0), stop=(i == NTD // 2 - 1),
                    perf_mode=DR)
        x0 = tmpC.tile([P, D], F32, tag="x0", name="x0")
        nc.vector.tensor_tensor(out=x0, in0=ps, in1=hst, op=ALU.add)
        x1t = x1_sb[:, ts, :]
        _layernorm(nc, tmpC, x1t, x0, g1_bc, b1_bc, eps_t,
                   badd_gpsimd=(ts % 2 == 0))
        if debug:
            nc.gpsimd.dma_start(
                out=dbg["dbg_x1"].rearrange("(t p) d -> p t d", p=P)[:, ts, :],
                in_=x1t)
        nc.sync.dma_start_transpose(out=x1T_sb[:, :, ts * P:(ts + 1) * P],
                                    in_=x1t)

    w1_pool = tc.alloc_tile_pool(name="w1p", bufs=2)
    hT_pool = tc.alloc_tile_pool(name="hTp", bufs=2)
    w2_pool = tc.alloc_tile_pool(name="w2p", bufs=1)
    out2_pool = tc.alloc_tile_pool(name="out2p", bufs=1)
    tmpE = tmpC
    _out2 = {}

    def out2_t(qg):
        if qg not in _out2:
            _out2.clear()
            _out2[qg] = out2_pool.tile([P, QTS, D], F32, tag="out2",
                                       name=f"out2_{qg}")
        return _out2[qg]

    _w1_cache = {}

    def fc1_group(g, qg, psF1):
        """fc1 for w1 stream group g (W1GT_ f-tiles) on query group qg.
        Returns hT quarter tile when it completes one."""
        if (g, qg) not in _w1_cache:
            w1 = w1_pool.tile([P, NTD, W1GT_ * P], BF16, tag="w1", name="w1")
            nc.sync.dma_start(out=w1, in_=f1w[g])
            _w1_cache.clear()
            _w1_cache[(g, qg)] = w1
        w1 = _w1_cache[(g, qg)]
        quarter = g // 2
        if g % 2 == 0:
            self_hT = hT_pool.tile([P, FQT, QW], BF16, tag="hT", name="hT")
            fc1_group.cur_hT = self_hT
        hT = fc1_group.cur_hT
        for ft in range(W1GT_):
            tf = g * W1GT_ + ft
            ps = psF1.tile([P, QW], F32, tag="u", name="psh")
            for td in range(NTD):
                nc.tensor.matmul(
                    ps,
                    lhsT=w1[:, td, ft * P:(ft + 1) * P],
                    rhs=x1T_sb[:, td, qg * QW:(qg + 1) * QW],
                    start=(td == 0), stop=(td == NTD - 1))
            nc.scalar.activation(
                out=hT[:, (tf % FQT), :], in_=ps, func=AFT.Gelu,
                bias=f1b_t[:, tf:tf + 1], scale=1.0)
        return hT if g % 2 == 1 else None

    def fc2_quarter(quarter, qg, hT):
        """fc2 for one F quarter on query group qg, accumulate into out2."""
        w2 = w2_pool.tile([P, FQT, D], BF16, tag="w2", name="w2")
        nc.scalar.dma_start(out=w2, in_=f2w[quarter])
        for tsq in range(QTS):
            ts = qg * QTS + tsq
            ps = psA.tile([P, D], F32, tag="u", name="pso2")
            for nch in range(NCH):
                for ft in range(FQT):
                    nc.tensor.matmul(
                        ps[:, nch * CW:(nch + 1) * CW],
                        lhsT=hT[:, ft, tsq * P:(tsq + 1) * P],
                        rhs=w2[:, ft, nch * CW:(nch + 1) * CW],
                        start=(ft == 0), stop=(ft == FQT - 1))
            if quarter == 0:
                nc.vector.tensor_tensor(out=out2_t(qg)[:, tsq, :], in0=ps,
                                        in1=f2b_bc, op=ALU.add)
            else:
                nc.vector.tensor_tensor(out=out2_t(qg)[:, tsq, :], in0=ps,
                                        in1=out2_t(qg)[:, tsq, :], op=ALU.add)
            if quarter == NFQ - 1:
                x0 = tmpE.tile([P, D], F32, tag="x0", name="x0e")
                nc.vector.tensor_tensor(out=x0, in0=out2_t(qg)[:, tsq, :],
                                        in1=x1_sb[:, ts, :], op=ALU.add)
                if debug:
                    nc.gpsimd.dma_start(
                        out=dbg["dbg_out2"].rearrange(
                            "(t p) d -> p t d", p=P)[:, ts, :],
                        in_=x0)
                yt = tmpE.tile([P, D], F32, tag="ye", name="ye")
                _layernorm(nc, tmpE, yt, x0, g2_bc, b2_bc, eps_t,
                           badd_gpsimd=(ts % 2 == 0))
                nc.sync.dma_start(out=out_r[:, ts, :], in_=yt)

    # ---------------- phase C: qg0 out-proj/FFN + qg1 attention ------------
    # out-projs of qg0 interleave with early qg1 scores so LN1 chains hide
    # under exp; fc1 groups fill PE once x1T(qg0) lands; one fc2 quarter
    # runs inside the h-loop.
    pend = None
    hT_done = []
    next_g = 0
    for h in range(H):
        if h < 2 * QTS and h % 2 == 0:
            out_proj_ln1(h // 2)
        prs = scores_head(h, 1)
        if pend is not None:
            attnv_head(pend[0], 1, pend[1])
        pend = (h, prs)
        if h >= 7 and h % 2 == 1 and next_g < 4:
            r = fc1_group(next_g, 0, psA)
            next_g += 1
            if r is not None:
                hT_done.append(r)
        if h == 14:
            fc2_quarter(0, 0, hT_done[0])
        if h == 15:
            r = fc1_group(next_g, 0, psA)
            next_g += 1
            if r is not None:
                hT_done.append(r)
    attnv_head(pend[0], 1, pend[1])
    in8b.release()

    r = fc1_group(5, 0, psA)
    hT_done.append(r)
    fc2_quarter(1, 0, hT_done[1])

    # ---------------- phase D: qg1 out-proj + FFN --------------------------
    # qg0's trailing fc1/fc2 work hides qg1's LN1 chains
    out_proj_ln1(QTS + 0)
    r = fc1_group(6, 0, psA)
    out_proj_ln1(QTS + 1)
    r = fc1_group(7, 0, psA)
    hT_done.append(r)
    out_proj_ln1(QTS + 2)
    fc2_quarter(2, 0, hT_done[2])
    out_proj_ln1(QTS + 3)
    fc2_quarter(3, 0, hT_done[3])
    in8c.release()

    if debug:
        for td in range(NTD):
            nc.gpsimd.dma_start(out=dbg["dbg_attnC"][td * P:(td + 1) * P, :],
                                in_=attnC[:, td, :])

    hT_done = []
    _w1_cache.clear()
    for g in range(NW1G_):
        r = fc1_group(g, 1, psA)
        if r is not None:
            hT_done.append(r)
        if g % 2 == 1:
            fc2_quarter(g // 2, 1, hT_done[g // 2])

    out2_pool.release()
    w2_pool.release()
    hT_pool.release()
    w1_pool.release()
    tmpC.release()
    hs_pool.release()
    lnp.release()
    rr_pool.release()
    bc_pool.release()
    probs_pool.release()
    psB.release()
    psA.release()
    x16.release()
    const.release()

def _layernorm(nc, pool, out_t, x0, g_bc, b_bc, eps_t, badd_gpsimd=False):
    """out = (x0 - mean)/sqrt(var+eps) * g + b   (mean/var along free dim D)"""
    stats = pool.tile([P, 2, 6], F32, tag="lnstats", name="lnstats")
    nc.vector.bn_stats(out=stats[:, 0, :], in_=x0[:, 0:512])
    nc.vector.bn_stats(out=stats[:, 1, :], in_=x0[:, 512:1024])
    mv = pool.tile([P, 2], F32, tag="lnmv", name="lnmv")
    nc.vector.bn_aggr(out=mv, in_=stats)
    nc.scalar.activation(out=mv[:, 1:2], in_=mv[:, 1:2], func=AFT.Sqrt,
                         bias=eps_t, scale=1.0)
    nc.vector.reciprocal(out=mv[:, 1:2], in_=mv[:, 1:2])
    # bneg = -mean * rstd, then xhat = x0*rstd + bneg on ACT (idle engine)
    bneg = pool.tile([P, 1], F32, tag="lnbneg", name="lnbneg")
    nc.vector.tensor_scalar(out=bneg, in0=mv[:, 0:1], scalar1=mv[:, 1:2],
                            scalar2=-1.0, op0=ALU.mult, op1=ALU.mult)
    nc.scalar.activation(out=out_t, in_=x0, func=AFT.Identity,
                         bias=bneg, scale=mv[:, 1:2])
    nc.gpsimd.tensor_tensor(out=out_t, in0=out_t, in1=g_bc, op=ALU.mult)
    badd = nc.gpsimd if badd_gpsimd else nc.vector
    badd.tensor_tensor(out=out_t, in0=out_t, in1=b_bc, op=ALU.add)


# ---------------------------------------------------------------------------
# Full-input entry point: data-parallel over batch across 8 NeuronCores.
# ---------------------------------------------------------------------------
import numpy as np
import ml_dtypes
from concourse import bass_utils

B = 8
SCALING = HD ** -0.5
E4 = ml_dtypes.float8_e4m3
BFNP = ml_dtypes.bfloat16

_NC_CACHE = None


def _get_nc():
    global _NC_CACHE
    if _NC_CACHE is None:
        _NC_CACHE = build(debug=False)
    return _NC_CACHE


def _prep_weights(w):
    c = np.ascontiguousarray
    f = np.float32

    def a(x):
        return c(np.asarray(x)).astype(f, copy=False)

    def q8(x):
        return c(np.asarray(x, dtype=np.float32).astype(E4))

    def q16(x):
        return c(np.asarray(x, dtype=np.float32).astype(BFNP))

    def tile_dt(x):
        # [D_in, C] -> [p, ti, c] with D_in = ti*128 + p
        return x.reshape(NTD, P, -1).transpose(1, 0, 2)

    wqT = np.asarray(w["wq"]).T * SCALING
    wkT = np.asarray(w["wk"]).T
    # wqk8[p, hp, j, ti, c] = wjT[ti*128+p, hp*128+c]
    wqk = np.stack([wqT, wkT], axis=0)           # [j, d_in, d_out]
    wqk = wqk.reshape(2, NTD, P, H // 2, P)      # [j, ti, p, hp, c]
    wqk = wqk.transpose(2, 3, 0, 1, 4)           # [p, hp, j, ti, c]

    f1wT = np.asarray(w["fc1_w"]).T              # [D, F]
    f1wt = f1wT.reshape(NTD, P, NW1G, W1GT * P)  # [ti, p, g, c]
    f1wt = f1wt.transpose(2, 1, 0, 3)            # [g, p, ti, c]

    f2wT = np.asarray(w["fc2_w"]).T              # [F, D]
    f2wt = f2wT.reshape(NFQ, FQT, P, D)          # [quarter, ft, p, c]
    f2wt = f2wt.transpose(0, 2, 1, 3)            # [quarter, p, ft, c]

    return {
        "wqk8": q8(wqk),
        "wv8": q8(tile_dt(np.asarray(w["wv"]).T)),
        "wo8": q8(tile_dt(np.asarray(w["wo"]).T)),
        "bqk": a(np.stack([np.asarray(w["bq"]) * SCALING, w["bk"]])),
        "bv": a(w["bv"]),
        "g1": a(w["ln1_g"]),
        "b1": a(w["ln1_b"]),
        "g2": a(w["ln2_g"]),
        "b2": a(w["ln2_b"]),
        "f1w": q16(f1wt),
        "f1b": a(w["fc1_b"]),
        "f2w": q16(f2wt),
        "f2b": a(w["fc2_b"]),
    }


def kernel(**inputs):
    """Takes full unsharded inputs (setup_inputs() keys), returns [B, S, D]."""
    w = {k: np.asarray(v) for k, v in inputs.items()}
    hs_all = w["hidden_states"]
    assert hs_all.shape == (B, S, D), hs_all.shape
    nc = _get_nc()
    wmap = _prep_weights(w)
    bo = np.asarray(w["bo"], dtype=np.float32)
    in_maps = []
    for c in range(B):
        b_hs = np.asarray(hs_all[c], dtype=np.float32)
        m = dict(wmap)
        hsT = b_hs.T.reshape(NTD, P, S).transpose(1, 0, 2)  # [p, td, s]
        m["hsT8"] = np.ascontiguousarray(hsT).astype(E4)
        hsb = (b_hs + bo[None, :]).reshape(NTS, P, D).transpose(1, 0, 2)
        m["hs"] = np.ascontiguousarray(hsb)
        in_maps.append(m)
    res = bass_utils.run_bass_kernel_spmd(nc, in_maps, core_ids=list(range(B)))
    out_full = np.stack([res.results[c]["out"] for c in range(B)])
    return out_full.astype(np.float32, copy=False)
